# revision 38
# baseline (speedup 1.0000x reference)
"""MoE FFN (8 experts, top-2, + shared expert) for 8 Trainium2 NeuronCores.

Strategy (expert-parallel, per sharding hint):
  - Host computes the (tiny) router: logits = x @ Wr.T, softmax, top-2,
    combine weights, aux loss.  This is 67 MFLOP of a ~155 GFLOP problem
    and is part of sharding/dispatch.
  - Token dispatch ("all-to-all") happens on the host while sharding:
    core e receives the tokens routed to expert e (gathered, padded to a
    uniform capacity) plus expert e's weights, and a 1/8 slice of tokens
    for the (replicated) shared expert.
  - Each core runs two SwiGLU-style FFNs in bf16 on the tensor engine:
    its expert over `cap` gathered tokens and the shared expert over
    T/8 tokens.  Activations are kept in "transposed" [D, tokens]
    layout so no on-device transposes are needed.
  - Host un-shards: scatter-adds the scaled expert outputs (each token
    appears in exactly two cores' outputs) and writes shared outputs.

All matmuls contract over the partition dim in 128-chunks; weights are
pre-transposed and pre-tiled on the host so every DMA is per-partition
contiguous.
"""

import math
import numpy as np
import ml_dtypes

BF16 = ml_dtypes.bfloat16

# Problem dims (nn_FFNwMoE: T tokens, D model, H hidden, E experts, top-2)
T, D, H, E, TOPK = 4096, 1024, 2048, 8, 2
NCORES = 8
P = 128
KD = D // P   # 8  k-tiles over D
KH = H // P   # 16 k-tiles over H
NS = T // NCORES  # shared-expert tokens per core (512)
AUX_COEF = 0.01

# module-level stash so test.py can read profiling info
LAST_RESULT = None
_NC_CACHE = {}


def _ensure_axon_ntff_hook():
    """concourse.bass_utils imports antenv.axon_hooks when tracing under
    axon; some images lack that module.  Provide a minimal equivalent
    backed by the injected libaxon_pjrt.so so profiling degrades
    gracefully instead of raising ImportError."""
    try:
        import antenv.axon_hooks  # noqa: F401
        return
    except ImportError:
        pass
    import sys
    import types
    import contextlib
    import ctypes
    import os

    mod = types.ModuleType("antenv.axon_hooks")
    state = {"hook": None}

    def set_axon_ntff_profile_hook(h):
        state["hook"] = h

    def get_axon_ntff_profile_hook():
        return state["hook"]

    mod.set_axon_ntff_profile_hook = set_axon_ntff_profile_hook
    mod.get_axon_ntff_profile_hook = get_axon_ntff_profile_hook

    so_path = "/opt/axon/libaxon_pjrt.so"
    if os.path.exists(so_path):
        try:
            lib = ctypes.CDLL(so_path)
            if hasattr(lib, "axon_start_nrt_profile"):
                lib.axon_start_nrt_profile.argtypes = [
                    ctypes.POINTER(ctypes.c_int64), ctypes.c_size_t]
                lib.axon_start_nrt_profile.restype = ctypes.c_int64
                lib.axon_stop_nrt_profile.argtypes = [ctypes.c_char_p]
                lib.axon_stop_nrt_profile.restype = ctypes.c_int64

                @contextlib.contextmanager
                def _hook(output_dir, device_ids):
                    started = False
                    try:
                        import jax
                        jax.devices()
                        if device_ids:
                            ids = (ctypes.c_int64 * len(device_ids))(*device_ids)
                            rc = lib.axon_start_nrt_profile(ids, len(device_ids))
                        else:
                            rc = lib.axon_start_nrt_profile(None, 0)
                        started = rc == 0
                    except Exception:
                        pass
                    try:
                        yield
                    finally:
                        if started:
                            try:
                                n = lib.axon_stop_nrt_profile(
                                    str(output_dir).encode())
                                print(f"ntff profile: {n} file(s) -> {output_dir}")
                            except Exception:
                                pass

                state["hook"] = _hook
        except OSError:
            pass

    try:
        import antenv
        sys.modules["antenv.axon_hooks"] = mod
        antenv.axon_hooks = mod
    except ImportError:
        pass


def _n_slices(ncols, step=512):
    # remainder tile first: the kernel then *ends* on long 512-col PSUM
    # groups whose matmul runs hide the copy/DMA drain latency.
    out = []
    c = 0
    while c < ncols:
        out.append((c, min(step, ncols - c)))
        c += step
    if len(out) > 1 and out[-1][1] < step:
        out = [out[-1]] + out[:-1]
        # recompute offsets so the small tile is at columns [0, rem)
        sizes = [s for _, s in out]
        out = []
        c = 0
        for s in sizes:
            out.append((c, s))
            c += s
    return out


def _build_bass(cap, act="Gelu"):
    import concourse.bacc as bacc
    import concourse.tile as tile
    import concourse.mybir as mybir
    from contextlib import ExitStack

    dt = mybir.dt
    nc = bacc.Bacc("TRN2", target_bir_lowering=False)

    # Per-core DRAM I/O.  Layouts chosen so every DMA is per-partition
    # contiguous:
    #   activations: [P, KD, ncols]   (= xT[k*128+p, col])
    #   up weights:  [KH, P, KD, 128] (m-tile major)
    #   down weights:[KD, P, KH, 128] (d-tile major)
    xg = nc.dram_tensor("xg", [P, KD, cap], dt.bfloat16, kind="ExternalInput")
    xs = nc.dram_tensor("xs", [P, KD, NS], dt.bfloat16, kind="ExternalInput")
    w1e = nc.dram_tensor("w1e", [KH, P, KD, P], dt.bfloat16, kind="ExternalInput")
    w3e = nc.dram_tensor("w3e", [KH, P, KD, P], dt.bfloat16, kind="ExternalInput")
    w2e = nc.dram_tensor("w2e", [KD, P, KH, P], dt.bfloat16, kind="ExternalInput")
    w1s = nc.dram_tensor("w1s", [KH, P, KD, P], dt.bfloat16, kind="ExternalInput")
    w3s = nc.dram_tensor("w3s", [KH, P, KD, P], dt.bfloat16, kind="ExternalInput")
    w2s = nc.dram_tensor("w2s", [KD, P, KH, P], dt.bfloat16, kind="ExternalInput")
    yg = nc.dram_tensor("yg", [KD, P, cap], dt.float32, kind="ExternalOutput")
    ys = nc.dram_tensor("ys", [KD, P, NS], dt.float32, kind="ExternalOutput")

    GELU = getattr(mybir.ActivationFunctionType, act)
    MULT = mybir.AluOpType.mult

    with ExitStack() as ctx:
        tc = ctx.enter_context(tile.TileContext(nc))
        xpool = ctx.enter_context(tc.tile_pool(name="xp", bufs=1))
        hpool = ctx.enter_context(tc.tile_pool(name="hp", bufs=1))
        wpool = ctx.enter_context(tc.tile_pool(name="wp", bufs=4))
        gpool = ctx.enter_context(tc.tile_pool(name="gp", bufs=4))
        opool = ctx.enter_context(tc.tile_pool(name="op", bufs=3))
        ps_up = ctx.enter_context(tc.tile_pool(name="ps_up", bufs=4, space="PSUM"))
        ps_dn = ctx.enter_context(tc.tile_pool(name="ps_dn", bufs=4, space="PSUM"))

        # PE warmup: ~20 matmuls on zeroed scratch with no DMA deps, so the
        # HAM clock-gate reaches 8/8 during the preamble + first DMA wait
        # instead of half-clocking the first ~12us of real matmuls.
        w_warm = gpool.tile([P, P], dt.bfloat16, tag="wwarm", name="w_warm")
        x_warm = gpool.tile([P, 512], dt.bfloat16, tag="xwarm", name="x_warm")
        nc.vector.memset(w_warm[:], 0.0)
        nc.vector.memset(x_warm[:], 0.0)
        for _ in range(24):
            ps_w = ps_dn.tile([P, 512], dt.float32, tag="psd", name="ps_w")
            nc.tensor.matmul(ps_w, w_warm[:], x_warm[:], start=True, stop=True)

        def ffn(x_dram, w1_dram, w3_dram, w2_dram, y_dram, ncols, sfx):
            nsl_up = _n_slices(ncols)
            nsl = _n_slices(ncols)
            # First m-tile's weights BEFORE the activations: DMA issue is
            # serialized (~0.6us each), so order determines time-to-first-MM.
            w1_t0 = wpool.tile([P, KD, P], dt.bfloat16, tag="w13", name="w1_t0")
            nc.sync.dma_start(out=w1_t0[:], in_=w1_dram[0])
            x_sb = xpool.tile([P, KD, ncols], dt.bfloat16, tag=f"x{sfx}")
            nc.sync.dma_start(out=x_sb[:], in_=x_dram[:])
            h_sb = hpool.tile([P, KH, ncols], dt.bfloat16, tag=f"h{sfx}")
            # ---- up/gate proj + gelu + mul -> h_sb
            for m in range(KH):
                if m == 0:
                    w1_t = w1_t0
                else:
                    w1_t = wpool.tile([P, KD, P], dt.bfloat16, tag="w13")
                    nc.sync.dma_start(out=w1_t[:], in_=w1_dram[m])
                w3_t = wpool.tile([P, KD, P], dt.bfloat16, tag="w13")
                nc.sync.dma_start(out=w3_t[:], in_=w3_dram[m])
                for (n0, nsz) in nsl_up:
                    ps1 = ps_up.tile([P, 512], dt.float32, tag="psu", name="ps1")[:, :nsz]
                    ps3 = ps_up.tile([P, 512], dt.float32, tag="psu", name="ps3")[:, :nsz]
                    for k in range(KD):
                        nc.tensor.matmul(
                            ps1, w1_t[:, k], x_sb[:, k, n0:n0 + nsz],
                            start=(k == 0), stop=(k == KD - 1),
                        )
                    for k in range(KD):
                        nc.tensor.matmul(
                            ps3, w3_t[:, k], x_sb[:, k, n0:n0 + nsz],
                            start=(k == 0), stop=(k == KD - 1),
                        )
                    g = gpool.tile([P, 512], dt.float32, tag="g", name="g")[:, :nsz]
                    nc.scalar.activation(g, ps1, GELU)
                    nc.vector.tensor_tensor(h_sb[:, m, n0:n0 + nsz], g, ps3, MULT)
            # ---- down proj -> y
            for d in range(KD):
                w2_t = wpool.tile([P, KH, P], dt.bfloat16, tag="w2")
                nc.sync.dma_start(out=w2_t[:], in_=w2_dram[d])
                for (n0, nsz) in nsl:
                    ps = ps_dn.tile([P, 512], dt.float32, tag="psd", name="ps")[:, :nsz]
                    for k in range(KH):
                        nc.tensor.matmul(
                            ps, w2_t[:, k], h_sb[:, k, n0:n0 + nsz],
                            start=(k == 0), stop=(k == KH - 1),
                        )
                    o = opool.tile([P, 512], dt.float32, tag="o", name="o")[:, :nsz]
                    nc.scalar.copy(o, ps)
                    nc.sync.dma_start(out=y_dram[d][:, n0:n0 + nsz], in_=o)

        # Shared expert first: it needs only ~1.3 MB of DMA before its
        # matmuls start, hiding the larger routed-expert loads behind it.
        ffn(xs, w1s, w3s, w2s, ys, NS, "s")
        ffn(xg, w1e, w3e, w2e, yg, cap, "g")

    nc.compile()  # bacc passes: split multi-waits into event semaphores etc.
    return nc


def _tile_up_w(wT):
    # wT: [D, H] fp32 -> [KH, P, KD, P] bf16  (W[m,p,k,c] = wT[k*128+p, m*128+c])
    return np.ascontiguousarray(
        wT.reshape(KD, P, KH, P).transpose(2, 1, 0, 3)
    ).astype(BF16)


def _tile_dn_w(wT):
    # wT: [H, D] fp32 -> [KD, P, KH, P] bf16  (W[d,p,k,c] = wT[k*128+p, d*128+c])
    return np.ascontiguousarray(
        wT.reshape(KH, P, KD, P).transpose(2, 1, 0, 3)
    ).astype(BF16)


def _tile_act(xT, ncols):
    # xT: [D, ncols] fp32 -> [P, KD, ncols] bf16
    return np.ascontiguousarray(
        xT.reshape(KD, P, ncols).transpose(1, 0, 2)
    ).astype(BF16)


def kernel(x, Wr, w1, w2, w3, w1s, w2s, w3s):
    global LAST_RESULT
    _ensure_axon_ntff_hook()
    from concourse.bass_utils import run_bass_kernel_spmd

    x = np.asarray(x, dtype=np.float32)
    Wr = np.asarray(Wr, dtype=np.float32)
    w1 = np.asarray(w1, dtype=np.float32)
    w2 = np.asarray(w2, dtype=np.float32)
    w3 = np.asarray(w3, dtype=np.float32)
    w1s = np.asarray(w1s, dtype=np.float32)
    w2s = np.asarray(w2s, dtype=np.float32)
    w3s = np.asarray(w3s, dtype=np.float32)

    # ---------------- host router (fp32, matches reference math) ----------
    logits = x @ Wr.T                                    # [T, E]
    m1 = logits.max(axis=1, keepdims=True)
    ex = np.exp(logits - m1)
    probs = ex / ex.sum(axis=1, keepdims=True)
    order = np.argsort(-logits, axis=1, kind="stable")   # top-k order
    top1, top2 = order[:, 0], order[:, 1]
    density = np.bincount(top1, minlength=E).astype(np.float32) / np.float32(T)
    aux = np.float32(AUX_COEF * float((density * probs.mean(0)).sum()) * E)

    idx = [np.flatnonzero((top1 == e) | (top2 == e)) for e in range(E)]
    counts = [len(i) for i in idx]
    cap = max(256, int(math.ceil(max(counts) / 16.0)) * 16)

    # ---------------- shard inputs per core --------------------------------
    xT = x.T  # [D, T]
    w1s_t = _tile_up_w(w1s.T)
    w3s_t = _tile_up_w(w3s.T)
    w2s_t = _tile_dn_w(w2s.T)

    in_maps = []
    for e in range(E):
        xgT = np.zeros((D, cap), np.float32)
        xgT[:, :counts[e]] = xT[:, idx[e]]
        xsT = xT[:, e * NS:(e + 1) * NS]
        in_maps.append(dict(
            xg=_tile_act(xgT, cap),
            xs=_tile_act(np.ascontiguousarray(xsT), NS),
            w1e=_tile_up_w(w1[e].T),
            w3e=_tile_up_w(w3[e].T),
            w2e=_tile_dn_w(w2[e].T),
            w1s=w1s_t, w3s=w3s_t, w2s=w2s_t,
        ))

    # ---------------- compile + run on 8 cores -----------------------------
    if cap not in _NC_CACHE:
        _NC_CACHE[cap] = _build_bass(cap)
    nc = _NC_CACHE[cap]
    res = None
    last_exc = None
    for attempt in range(3):  # transient NRT exec-unit errors recover on retry
        try:
            res = run_bass_kernel_spmd(nc, in_maps, core_ids=list(range(NCORES)))
            break
        except Exception as e:  # noqa: BLE001
            last_exc = e
            import time
            time.sleep(2.0)
    if res is None:
        raise last_exc
    LAST_RESULT = res

    # ---------------- host un-shard / combine ------------------------------
    out = np.empty((T, D), np.float32)
    for e in range(E):
        ysT = res.results[e]["ys"].reshape(D, NS)        # [D, NS]
        out[e * NS:(e + 1) * NS] = ysT.T
    for e in range(E):
        ygT = res.results[e]["yg"].reshape(D, cap)       # [D, cap]
        ye = ygT.T[:counts[e]]                           # [cnt, D]
        out[idx[e]] += probs[idx[e], e][:, None] * ye
    return out, aux


# revision 39
# speedup vs baseline: 1.0048x; 1.0048x over previous
"""MoE FFN (8 experts, top-2, + shared expert) for 8 Trainium2 NeuronCores.

Strategy (expert-parallel, per sharding hint):
  - Host computes the (tiny) router: logits = x @ Wr.T, softmax, top-2,
    combine weights, aux loss.  This is 67 MFLOP of a ~155 GFLOP problem
    and is part of sharding/dispatch.
  - Token dispatch ("all-to-all") happens on the host while sharding:
    core e receives the tokens routed to expert e (gathered, padded to a
    uniform capacity) plus expert e's weights, and a 1/8 slice of tokens
    for the (replicated) shared expert.
  - Each core runs two SwiGLU-style FFNs in bf16 on the tensor engine:
    its expert over `cap` gathered tokens and the shared expert over
    T/8 tokens.  Activations are kept in "transposed" [D, tokens]
    layout so no on-device transposes are needed.
  - Host un-shards: scatter-adds the scaled expert outputs (each token
    appears in exactly two cores' outputs) and writes shared outputs.

All matmuls contract over the partition dim in 128-chunks; weights are
pre-transposed and pre-tiled on the host so every DMA is per-partition
contiguous.
"""

import math
import numpy as np
import ml_dtypes

BF16 = ml_dtypes.bfloat16

# Problem dims (nn_FFNwMoE: T tokens, D model, H hidden, E experts, top-2)
T, D, H, E, TOPK = 4096, 1024, 2048, 8, 2
NCORES = 8
P = 128
KD = D // P   # 8  k-tiles over D
KH = H // P   # 16 k-tiles over H
NS = T // NCORES  # shared-expert tokens per core (512)
AUX_COEF = 0.01

# module-level stash so test.py can read profiling info
LAST_RESULT = None
_NC_CACHE = {}


def _ensure_axon_ntff_hook():
    """concourse.bass_utils imports antenv.axon_hooks when tracing under
    axon; some images lack that module.  Provide a minimal equivalent
    backed by the injected libaxon_pjrt.so so profiling degrades
    gracefully instead of raising ImportError."""
    try:
        import antenv.axon_hooks  # noqa: F401
        return
    except ImportError:
        pass
    import sys
    import types
    import contextlib
    import ctypes
    import os

    mod = types.ModuleType("antenv.axon_hooks")
    state = {"hook": None}

    def set_axon_ntff_profile_hook(h):
        state["hook"] = h

    def get_axon_ntff_profile_hook():
        return state["hook"]

    mod.set_axon_ntff_profile_hook = set_axon_ntff_profile_hook
    mod.get_axon_ntff_profile_hook = get_axon_ntff_profile_hook

    so_path = "/opt/axon/libaxon_pjrt.so"
    if os.path.exists(so_path):
        try:
            lib = ctypes.CDLL(so_path)
            if hasattr(lib, "axon_start_nrt_profile"):
                lib.axon_start_nrt_profile.argtypes = [
                    ctypes.POINTER(ctypes.c_int64), ctypes.c_size_t]
                lib.axon_start_nrt_profile.restype = ctypes.c_int64
                lib.axon_stop_nrt_profile.argtypes = [ctypes.c_char_p]
                lib.axon_stop_nrt_profile.restype = ctypes.c_int64

                @contextlib.contextmanager
                def _hook(output_dir, device_ids):
                    started = False
                    try:
                        import jax
                        jax.devices()
                        if device_ids:
                            ids = (ctypes.c_int64 * len(device_ids))(*device_ids)
                            rc = lib.axon_start_nrt_profile(ids, len(device_ids))
                        else:
                            rc = lib.axon_start_nrt_profile(None, 0)
                        started = rc == 0
                    except Exception:
                        pass
                    try:
                        yield
                    finally:
                        if started:
                            try:
                                n = lib.axon_stop_nrt_profile(
                                    str(output_dir).encode())
                                print(f"ntff profile: {n} file(s) -> {output_dir}")
                            except Exception:
                                pass

                state["hook"] = _hook
        except OSError:
            pass

    try:
        import antenv
        sys.modules["antenv.axon_hooks"] = mod
        antenv.axon_hooks = mod
    except ImportError:
        pass


def _n_slices(ncols, step=512):
    # remainder tile first: the kernel then *ends* on long 512-col PSUM
    # groups whose matmul runs hide the copy/DMA drain latency.
    out = []
    c = 0
    while c < ncols:
        out.append((c, min(step, ncols - c)))
        c += step
    if len(out) > 1 and out[-1][1] < step:
        out = [out[-1]] + out[:-1]
        # recompute offsets so the small tile is at columns [0, rem)
        sizes = [s for _, s in out]
        out = []
        c = 0
        for s in sizes:
            out.append((c, s))
            c += s
    return out


def _build_bass(cap, act="Gelu"):
    import concourse.bacc as bacc
    import concourse.tile as tile
    import concourse.mybir as mybir
    from contextlib import ExitStack

    dt = mybir.dt
    nc = bacc.Bacc("TRN2", target_bir_lowering=False)

    # Per-core DRAM I/O.  Layouts chosen so every DMA is per-partition
    # contiguous:
    #   activations: [P, KD, ncols]   (= xT[k*128+p, col])
    #   up weights:  [KH, P, KD, 128] (m-tile major)
    #   down weights:[KD, P, KH, 128] (d-tile major)
    xg = nc.dram_tensor("xg", [P, KD, cap], dt.bfloat16, kind="ExternalInput")
    xs = nc.dram_tensor("xs", [P, KD, NS], dt.bfloat16, kind="ExternalInput")
    w1e = nc.dram_tensor("w1e", [KH, P, KD, P], dt.bfloat16, kind="ExternalInput")
    w3e = nc.dram_tensor("w3e", [KH, P, KD, P], dt.bfloat16, kind="ExternalInput")
    w2e = nc.dram_tensor("w2e", [KD, P, KH, P], dt.bfloat16, kind="ExternalInput")
    w1s = nc.dram_tensor("w1s", [KH, P, KD, P], dt.bfloat16, kind="ExternalInput")
    w3s = nc.dram_tensor("w3s", [KH, P, KD, P], dt.bfloat16, kind="ExternalInput")
    w2s = nc.dram_tensor("w2s", [KD, P, KH, P], dt.bfloat16, kind="ExternalInput")
    yg = nc.dram_tensor("yg", [KD, P, cap], dt.float32, kind="ExternalOutput")
    ys = nc.dram_tensor("ys", [KD, P, NS], dt.float32, kind="ExternalOutput")

    GELU = getattr(mybir.ActivationFunctionType, act)
    MULT = mybir.AluOpType.mult

    with ExitStack() as ctx:
        tc = ctx.enter_context(tile.TileContext(nc))
        xpool = ctx.enter_context(tc.tile_pool(name="xp", bufs=1))
        hpool = ctx.enter_context(tc.tile_pool(name="hp", bufs=1))
        wpool = ctx.enter_context(tc.tile_pool(name="wp", bufs=6))
        gpool = ctx.enter_context(tc.tile_pool(name="gp", bufs=4))
        opool = ctx.enter_context(tc.tile_pool(name="op", bufs=3))
        ps_up = ctx.enter_context(tc.tile_pool(name="ps_up", bufs=4, space="PSUM"))
        ps_dn = ctx.enter_context(tc.tile_pool(name="ps_dn", bufs=4, space="PSUM"))

        # PE warmup: ~20 matmuls on zeroed scratch with no DMA deps, so the
        # HAM clock-gate reaches 8/8 during the preamble + first DMA wait
        # instead of half-clocking the first ~12us of real matmuls.
        w_warm = gpool.tile([P, P], dt.bfloat16, tag="wwarm", name="w_warm")
        x_warm = gpool.tile([P, 512], dt.bfloat16, tag="xwarm", name="x_warm")
        nc.vector.memset(w_warm[:], 0.0)
        nc.vector.memset(x_warm[:], 0.0)
        for _ in range(24):
            ps_w = ps_dn.tile([P, 512], dt.float32, tag="psd", name="ps_w")
            nc.tensor.matmul(ps_w, w_warm[:], x_warm[:], start=True, stop=True)

        def ffn(x_dram, w1_dram, w3_dram, w2_dram, y_dram, ncols, sfx):
            nsl_up = _n_slices(ncols)
            nsl = _n_slices(ncols)
            # First m-tile's weights BEFORE the activations: DMA issue is
            # serialized (~0.6us each), so order determines time-to-first-MM.
            w1_t0 = wpool.tile([P, KD, P], dt.bfloat16, tag="w13", name="w1_t0")
            nc.sync.dma_start(out=w1_t0[:], in_=w1_dram[0])
            x_sb = xpool.tile([P, KD, ncols], dt.bfloat16, tag=f"x{sfx}")
            nc.sync.dma_start(out=x_sb[:], in_=x_dram[:])
            h_sb = hpool.tile([P, KH, ncols], dt.bfloat16, tag=f"h{sfx}")
            # ---- up/gate proj + gelu + mul -> h_sb
            for m in range(KH):
                if m == 0:
                    w1_t = w1_t0
                else:
                    w1_t = wpool.tile([P, KD, P], dt.bfloat16, tag="w13")
                    nc.sync.dma_start(out=w1_t[:], in_=w1_dram[m])
                w3_t = wpool.tile([P, KD, P], dt.bfloat16, tag="w13")
                nc.sync.dma_start(out=w3_t[:], in_=w3_dram[m])
                for (n0, nsz) in nsl_up:
                    ps1 = ps_up.tile([P, 512], dt.float32, tag="psu", name="ps1")[:, :nsz]
                    ps3 = ps_up.tile([P, 512], dt.float32, tag="psu", name="ps3")[:, :nsz]
                    for k in range(KD):
                        nc.tensor.matmul(
                            ps1, w1_t[:, k], x_sb[:, k, n0:n0 + nsz],
                            start=(k == 0), stop=(k == KD - 1),
                        )
                    for k in range(KD):
                        nc.tensor.matmul(
                            ps3, w3_t[:, k], x_sb[:, k, n0:n0 + nsz],
                            start=(k == 0), stop=(k == KD - 1),
                        )
                    g = gpool.tile([P, 512], dt.float32, tag="g", name="g")[:, :nsz]
                    nc.scalar.activation(g, ps1, GELU)
                    nc.vector.tensor_tensor(h_sb[:, m, n0:n0 + nsz], g, ps3, MULT)
            # ---- down proj -> y
            for d in range(KD):
                w2_t = wpool.tile([P, KH, P], dt.bfloat16, tag="w2")
                nc.sync.dma_start(out=w2_t[:], in_=w2_dram[d])
                for (n0, nsz) in nsl:
                    ps = ps_dn.tile([P, 512], dt.float32, tag="psd", name="ps")[:, :nsz]
                    for k in range(KH):
                        nc.tensor.matmul(
                            ps, w2_t[:, k], h_sb[:, k, n0:n0 + nsz],
                            start=(k == 0), stop=(k == KH - 1),
                        )
                    o = opool.tile([P, 512], dt.float32, tag="o", name="o")[:, :nsz]
                    nc.scalar.copy(o, ps)
                    nc.sync.dma_start(out=y_dram[d][:, n0:n0 + nsz], in_=o)

        # Shared expert first: it needs only ~1.3 MB of DMA before its
        # matmuls start, hiding the larger routed-expert loads behind it.
        ffn(xs, w1s, w3s, w2s, ys, NS, "s")
        ffn(xg, w1e, w3e, w2e, yg, cap, "g")

    nc.compile()  # bacc passes: split multi-waits into event semaphores etc.
    return nc


def _tile_up_w(wT):
    # wT: [D, H] fp32 -> [KH, P, KD, P] bf16  (W[m,p,k,c] = wT[k*128+p, m*128+c])
    return np.ascontiguousarray(
        wT.reshape(KD, P, KH, P).transpose(2, 1, 0, 3)
    ).astype(BF16)


def _tile_dn_w(wT):
    # wT: [H, D] fp32 -> [KD, P, KH, P] bf16  (W[d,p,k,c] = wT[k*128+p, d*128+c])
    return np.ascontiguousarray(
        wT.reshape(KH, P, KD, P).transpose(2, 1, 0, 3)
    ).astype(BF16)


def _tile_act(xT, ncols):
    # xT: [D, ncols] fp32 -> [P, KD, ncols] bf16
    return np.ascontiguousarray(
        xT.reshape(KD, P, ncols).transpose(1, 0, 2)
    ).astype(BF16)


def kernel(x, Wr, w1, w2, w3, w1s, w2s, w3s):
    global LAST_RESULT
    _ensure_axon_ntff_hook()
    from concourse.bass_utils import run_bass_kernel_spmd

    x = np.asarray(x, dtype=np.float32)
    Wr = np.asarray(Wr, dtype=np.float32)
    w1 = np.asarray(w1, dtype=np.float32)
    w2 = np.asarray(w2, dtype=np.float32)
    w3 = np.asarray(w3, dtype=np.float32)
    w1s = np.asarray(w1s, dtype=np.float32)
    w2s = np.asarray(w2s, dtype=np.float32)
    w3s = np.asarray(w3s, dtype=np.float32)

    # ---------------- host router (fp32, matches reference math) ----------
    logits = x @ Wr.T                                    # [T, E]
    m1 = logits.max(axis=1, keepdims=True)
    ex = np.exp(logits - m1)
    probs = ex / ex.sum(axis=1, keepdims=True)
    order = np.argsort(-logits, axis=1, kind="stable")   # top-k order
    top1, top2 = order[:, 0], order[:, 1]
    density = np.bincount(top1, minlength=E).astype(np.float32) / np.float32(T)
    aux = np.float32(AUX_COEF * float((density * probs.mean(0)).sum()) * E)

    idx = [np.flatnonzero((top1 == e) | (top2 == e)) for e in range(E)]
    counts = [len(i) for i in idx]
    cap = max(256, int(math.ceil(max(counts) / 16.0)) * 16)

    # ---------------- shard inputs per core --------------------------------
    xT = x.T  # [D, T]
    w1s_t = _tile_up_w(w1s.T)
    w3s_t = _tile_up_w(w3s.T)
    w2s_t = _tile_dn_w(w2s.T)

    in_maps = []
    for e in range(E):
        xgT = np.zeros((D, cap), np.float32)
        xgT[:, :counts[e]] = xT[:, idx[e]]
        xsT = xT[:, e * NS:(e + 1) * NS]
        in_maps.append(dict(
            xg=_tile_act(xgT, cap),
            xs=_tile_act(np.ascontiguousarray(xsT), NS),
            w1e=_tile_up_w(w1[e].T),
            w3e=_tile_up_w(w3[e].T),
            w2e=_tile_dn_w(w2[e].T),
            w1s=w1s_t, w3s=w3s_t, w2s=w2s_t,
        ))

    # ---------------- compile + run on 8 cores -----------------------------
    if cap not in _NC_CACHE:
        _NC_CACHE[cap] = _build_bass(cap)
    nc = _NC_CACHE[cap]
    res = None
    last_exc = None
    for attempt in range(3):  # transient NRT exec-unit errors recover on retry
        try:
            res = run_bass_kernel_spmd(nc, in_maps, core_ids=list(range(NCORES)))
            break
        except Exception as e:  # noqa: BLE001
            last_exc = e
            import time
            time.sleep(2.0)
    if res is None:
        raise last_exc
    LAST_RESULT = res

    # ---------------- host un-shard / combine ------------------------------
    out = np.empty((T, D), np.float32)
    for e in range(E):
        ysT = res.results[e]["ys"].reshape(D, NS)        # [D, NS]
        out[e * NS:(e + 1) * NS] = ysT.T
    for e in range(E):
        ygT = res.results[e]["yg"].reshape(D, cap)       # [D, cap]
        ye = ygT.T[:counts[e]]                           # [cnt, D]
        out[idx[e]] += probs[idx[e], e][:, None] * ye
    return out, aux


# revision 40
# speedup vs baseline: 1.0199x; 1.0151x over previous
"""MoE FFN (8 experts, top-2, + shared expert) for 8 Trainium2 NeuronCores.

Strategy (expert-parallel, per sharding hint):
  - Host computes the (tiny) router: logits = x @ Wr.T, softmax, top-2,
    combine weights, aux loss.  This is 67 MFLOP of a ~155 GFLOP problem
    and is part of sharding/dispatch.
  - Token dispatch ("all-to-all") happens on the host while sharding:
    core e receives the tokens routed to expert e (gathered, padded to a
    uniform capacity) plus expert e's weights, and a 1/8 slice of tokens
    for the (replicated) shared expert.
  - Each core runs two SwiGLU-style FFNs in bf16 on the tensor engine:
    its expert over `cap` gathered tokens and the shared expert over
    T/8 tokens.  Activations are kept in "transposed" [D, tokens]
    layout so no on-device transposes are needed.
  - Host un-shards: scatter-adds the scaled expert outputs (each token
    appears in exactly two cores' outputs) and writes shared outputs.

All matmuls contract over the partition dim in 128-chunks; weights are
pre-transposed and pre-tiled on the host so every DMA is per-partition
contiguous.
"""

import math
import numpy as np
import ml_dtypes

BF16 = ml_dtypes.bfloat16

# Problem dims (nn_FFNwMoE: T tokens, D model, H hidden, E experts, top-2)
T, D, H, E, TOPK = 4096, 1024, 2048, 8, 2
NCORES = 8
P = 128
KD = D // P   # 8  k-tiles over D
KH = H // P   # 16 k-tiles over H
NS = T // NCORES  # shared-expert tokens per core (512)
AUX_COEF = 0.01

# module-level stash so test.py can read profiling info
LAST_RESULT = None
_NC_CACHE = {}


def _ensure_axon_ntff_hook():
    """concourse.bass_utils imports antenv.axon_hooks when tracing under
    axon; some images lack that module.  Provide a minimal equivalent
    backed by the injected libaxon_pjrt.so so profiling degrades
    gracefully instead of raising ImportError."""
    try:
        import antenv.axon_hooks  # noqa: F401
        return
    except ImportError:
        pass
    import sys
    import types
    import contextlib
    import ctypes
    import os

    mod = types.ModuleType("antenv.axon_hooks")
    state = {"hook": None}

    def set_axon_ntff_profile_hook(h):
        state["hook"] = h

    def get_axon_ntff_profile_hook():
        return state["hook"]

    mod.set_axon_ntff_profile_hook = set_axon_ntff_profile_hook
    mod.get_axon_ntff_profile_hook = get_axon_ntff_profile_hook

    so_path = "/opt/axon/libaxon_pjrt.so"
    if os.path.exists(so_path):
        try:
            lib = ctypes.CDLL(so_path)
            if hasattr(lib, "axon_start_nrt_profile"):
                lib.axon_start_nrt_profile.argtypes = [
                    ctypes.POINTER(ctypes.c_int64), ctypes.c_size_t]
                lib.axon_start_nrt_profile.restype = ctypes.c_int64
                lib.axon_stop_nrt_profile.argtypes = [ctypes.c_char_p]
                lib.axon_stop_nrt_profile.restype = ctypes.c_int64

                @contextlib.contextmanager
                def _hook(output_dir, device_ids):
                    started = False
                    try:
                        import jax
                        jax.devices()
                        if device_ids:
                            ids = (ctypes.c_int64 * len(device_ids))(*device_ids)
                            rc = lib.axon_start_nrt_profile(ids, len(device_ids))
                        else:
                            rc = lib.axon_start_nrt_profile(None, 0)
                        started = rc == 0
                    except Exception:
                        pass
                    try:
                        yield
                    finally:
                        if started:
                            try:
                                n = lib.axon_stop_nrt_profile(
                                    str(output_dir).encode())
                                print(f"ntff profile: {n} file(s) -> {output_dir}")
                            except Exception:
                                pass

                state["hook"] = _hook
        except OSError:
            pass

    try:
        import antenv
        sys.modules["antenv.axon_hooks"] = mod
        antenv.axon_hooks = mod
    except ImportError:
        pass


def _n_slices(ncols, step=512):
    # remainder tile first: the kernel then *ends* on long 512-col PSUM
    # groups whose matmul runs hide the copy/DMA drain latency.
    out = []
    c = 0
    while c < ncols:
        out.append((c, min(step, ncols - c)))
        c += step
    if len(out) > 1 and out[-1][1] < step:
        out = [out[-1]] + out[:-1]
        # recompute offsets so the small tile is at columns [0, rem)
        sizes = [s for _, s in out]
        out = []
        c = 0
        for s in sizes:
            out.append((c, s))
            c += s
    return out


def _build_bass(cap, act="Gelu"):
    import concourse.bacc as bacc
    import concourse.tile as tile
    import concourse.mybir as mybir
    from contextlib import ExitStack

    dt = mybir.dt
    nc = bacc.Bacc("TRN2", target_bir_lowering=False)

    # Per-core DRAM I/O.  Layouts chosen so every DMA is per-partition
    # contiguous:
    #   activations: [P, KD, ncols]   (= xT[k*128+p, col])
    #   up weights:  [KH, P, KD, 128] (m-tile major)
    #   down weights:[KD, P, KH, 128] (d-tile major)
    xg = nc.dram_tensor("xg", [P, KD, cap], dt.bfloat16, kind="ExternalInput")
    xs = nc.dram_tensor("xs", [P, KD, NS], dt.bfloat16, kind="ExternalInput")
    w1e = nc.dram_tensor("w1e", [KH, P, KD, P], dt.bfloat16, kind="ExternalInput")
    w3e = nc.dram_tensor("w3e", [KH, P, KD, P], dt.bfloat16, kind="ExternalInput")
    w2e = nc.dram_tensor("w2e", [KD, P, KH, P], dt.bfloat16, kind="ExternalInput")
    w1s = nc.dram_tensor("w1s", [KH, P, KD, P], dt.bfloat16, kind="ExternalInput")
    w3s = nc.dram_tensor("w3s", [KH, P, KD, P], dt.bfloat16, kind="ExternalInput")
    w2s = nc.dram_tensor("w2s", [KD, P, KH, P], dt.bfloat16, kind="ExternalInput")
    yg = nc.dram_tensor("yg", [KD, P, cap], dt.float32, kind="ExternalOutput")
    ys = nc.dram_tensor("ys", [KD, P, NS], dt.float32, kind="ExternalOutput")

    GELU = getattr(mybir.ActivationFunctionType, act)
    MULT = mybir.AluOpType.mult

    with ExitStack() as ctx:
        tc = ctx.enter_context(tile.TileContext(nc))
        xpool = ctx.enter_context(tc.tile_pool(name="xp", bufs=1))
        hpool = ctx.enter_context(tc.tile_pool(name="hp", bufs=1))
        wpool = ctx.enter_context(tc.tile_pool(name="wp", bufs=10))
        gpool = ctx.enter_context(tc.tile_pool(name="gp", bufs=4))
        opool = ctx.enter_context(tc.tile_pool(name="op", bufs=3))
        ps_up = ctx.enter_context(tc.tile_pool(name="ps_up", bufs=4, space="PSUM"))
        ps_dn = ctx.enter_context(tc.tile_pool(name="ps_dn", bufs=4, space="PSUM"))

        # PE warmup: ~20 matmuls on zeroed scratch with no DMA deps, so the
        # HAM clock-gate reaches 8/8 during the preamble + first DMA wait
        # instead of half-clocking the first ~12us of real matmuls.
        w_warm = gpool.tile([P, P], dt.bfloat16, tag="wwarm", name="w_warm")
        x_warm = gpool.tile([P, 512], dt.bfloat16, tag="xwarm", name="x_warm")
        nc.vector.memset(w_warm[:], 0.0)
        nc.vector.memset(x_warm[:], 0.0)
        for _ in range(24):
            ps_w = ps_dn.tile([P, 512], dt.float32, tag="psd", name="ps_w")
            nc.tensor.matmul(ps_w, w_warm[:], x_warm[:], start=True, stop=True)

        def ffn(x_dram, w1_dram, w3_dram, w2_dram, y_dram, ncols, sfx):
            nsl_up = _n_slices(ncols)
            nsl = _n_slices(ncols)
            # First m-tile's weights BEFORE the activations: DMA issue is
            # serialized (~0.6us each), so order determines time-to-first-MM.
            w1_t0 = wpool.tile([P, KD, P], dt.bfloat16, tag="w13", name="w1_t0")
            nc.sync.dma_start(out=w1_t0[:], in_=w1_dram[0])
            x_sb = xpool.tile([P, KD, ncols], dt.bfloat16, tag=f"x{sfx}")
            nc.sync.dma_start(out=x_sb[:], in_=x_dram[:])
            h_sb = hpool.tile([P, KH, ncols], dt.bfloat16, tag=f"h{sfx}")
            # ---- up/gate proj + gelu + mul -> h_sb
            for m in range(KH):
                if m == 0:
                    w1_t = w1_t0
                else:
                    w1_t = wpool.tile([P, KD, P], dt.bfloat16, tag="w13")
                    nc.sync.dma_start(out=w1_t[:], in_=w1_dram[m])
                w3_t = wpool.tile([P, KD, P], dt.bfloat16, tag="w13")
                nc.sync.dma_start(out=w3_t[:], in_=w3_dram[m])
                for (n0, nsz) in nsl_up:
                    ps1 = ps_up.tile([P, 512], dt.float32, tag="psu", name="ps1")[:, :nsz]
                    ps3 = ps_up.tile([P, 512], dt.float32, tag="psu", name="ps3")[:, :nsz]
                    for k in range(KD):
                        nc.tensor.matmul(
                            ps1, w1_t[:, k], x_sb[:, k, n0:n0 + nsz],
                            start=(k == 0), stop=(k == KD - 1),
                        )
                    for k in range(KD):
                        nc.tensor.matmul(
                            ps3, w3_t[:, k], x_sb[:, k, n0:n0 + nsz],
                            start=(k == 0), stop=(k == KD - 1),
                        )
                    g = gpool.tile([P, 512], dt.float32, tag="g", name="g")[:, :nsz]
                    nc.scalar.activation(g, ps1, GELU)
                    nc.vector.tensor_tensor(h_sb[:, m, n0:n0 + nsz], g, ps3, MULT)
            # ---- down proj -> y
            for d in range(KD):
                w2_t = wpool.tile([P, KH, P], dt.bfloat16, tag="w2")
                nc.sync.dma_start(out=w2_t[:], in_=w2_dram[d])
                for (n0, nsz) in nsl:
                    ps = ps_dn.tile([P, 512], dt.float32, tag="psd", name="ps")[:, :nsz]
                    for k in range(KH):
                        nc.tensor.matmul(
                            ps, w2_t[:, k], h_sb[:, k, n0:n0 + nsz],
                            start=(k == 0), stop=(k == KH - 1),
                        )
                    o = opool.tile([P, 512], dt.float32, tag="o", name="o")[:, :nsz]
                    nc.scalar.copy(o, ps)
                    nc.sync.dma_start(out=y_dram[d][:, n0:n0 + nsz], in_=o)

        # Shared expert first: it needs only ~1.3 MB of DMA before its
        # matmuls start, hiding the larger routed-expert loads behind it.
        ffn(xs, w1s, w3s, w2s, ys, NS, "s")
        ffn(xg, w1e, w3e, w2e, yg, cap, "g")

    nc.compile()  # bacc passes: split multi-waits into event semaphores etc.
    return nc


def _tile_up_w(wT):
    # wT: [D, H] fp32 -> [KH, P, KD, P] bf16  (W[m,p,k,c] = wT[k*128+p, m*128+c])
    return np.ascontiguousarray(
        wT.reshape(KD, P, KH, P).transpose(2, 1, 0, 3)
    ).astype(BF16)


def _tile_dn_w(wT):
    # wT: [H, D] fp32 -> [KD, P, KH, P] bf16  (W[d,p,k,c] = wT[k*128+p, d*128+c])
    return np.ascontiguousarray(
        wT.reshape(KH, P, KD, P).transpose(2, 1, 0, 3)
    ).astype(BF16)


def _tile_act(xT, ncols):
    # xT: [D, ncols] fp32 -> [P, KD, ncols] bf16
    return np.ascontiguousarray(
        xT.reshape(KD, P, ncols).transpose(1, 0, 2)
    ).astype(BF16)


def kernel(x, Wr, w1, w2, w3, w1s, w2s, w3s):
    global LAST_RESULT
    _ensure_axon_ntff_hook()
    from concourse.bass_utils import run_bass_kernel_spmd

    x = np.asarray(x, dtype=np.float32)
    Wr = np.asarray(Wr, dtype=np.float32)
    w1 = np.asarray(w1, dtype=np.float32)
    w2 = np.asarray(w2, dtype=np.float32)
    w3 = np.asarray(w3, dtype=np.float32)
    w1s = np.asarray(w1s, dtype=np.float32)
    w2s = np.asarray(w2s, dtype=np.float32)
    w3s = np.asarray(w3s, dtype=np.float32)

    # ---------------- host router (fp32, matches reference math) ----------
    logits = x @ Wr.T                                    # [T, E]
    m1 = logits.max(axis=1, keepdims=True)
    ex = np.exp(logits - m1)
    probs = ex / ex.sum(axis=1, keepdims=True)
    order = np.argsort(-logits, axis=1, kind="stable")   # top-k order
    top1, top2 = order[:, 0], order[:, 1]
    density = np.bincount(top1, minlength=E).astype(np.float32) / np.float32(T)
    aux = np.float32(AUX_COEF * float((density * probs.mean(0)).sum()) * E)

    idx = [np.flatnonzero((top1 == e) | (top2 == e)) for e in range(E)]
    counts = [len(i) for i in idx]
    cap = max(256, int(math.ceil(max(counts) / 16.0)) * 16)

    # ---------------- shard inputs per core --------------------------------
    xT = x.T  # [D, T]
    w1s_t = _tile_up_w(w1s.T)
    w3s_t = _tile_up_w(w3s.T)
    w2s_t = _tile_dn_w(w2s.T)

    in_maps = []
    for e in range(E):
        xgT = np.zeros((D, cap), np.float32)
        xgT[:, :counts[e]] = xT[:, idx[e]]
        xsT = xT[:, e * NS:(e + 1) * NS]
        in_maps.append(dict(
            xg=_tile_act(xgT, cap),
            xs=_tile_act(np.ascontiguousarray(xsT), NS),
            w1e=_tile_up_w(w1[e].T),
            w3e=_tile_up_w(w3[e].T),
            w2e=_tile_dn_w(w2[e].T),
            w1s=w1s_t, w3s=w3s_t, w2s=w2s_t,
        ))

    # ---------------- compile + run on 8 cores -----------------------------
    if cap not in _NC_CACHE:
        _NC_CACHE[cap] = _build_bass(cap)
    nc = _NC_CACHE[cap]
    res = None
    last_exc = None
    for attempt in range(3):  # transient NRT exec-unit errors recover on retry
        try:
            res = run_bass_kernel_spmd(nc, in_maps, core_ids=list(range(NCORES)))
            break
        except Exception as e:  # noqa: BLE001
            last_exc = e
            import time
            time.sleep(2.0)
    if res is None:
        raise last_exc
    LAST_RESULT = res

    # ---------------- host un-shard / combine ------------------------------
    out = np.empty((T, D), np.float32)
    for e in range(E):
        ysT = res.results[e]["ys"].reshape(D, NS)        # [D, NS]
        out[e * NS:(e + 1) * NS] = ysT.T
    for e in range(E):
        ygT = res.results[e]["yg"].reshape(D, cap)       # [D, cap]
        ye = ygT.T[:counts[e]]                           # [cnt, D]
        out[idx[e]] += probs[idx[e], e][:, None] * ye
    return out, aux


# revision 41
# speedup vs baseline: 1.0241x; 1.0041x over previous
"""MoE FFN (8 experts, top-2, + shared expert) for 8 Trainium2 NeuronCores.

Strategy (expert-parallel, per sharding hint):
  - Host computes the (tiny) router: logits = x @ Wr.T, softmax, top-2,
    combine weights, aux loss.  This is 67 MFLOP of a ~155 GFLOP problem
    and is part of sharding/dispatch.
  - Token dispatch ("all-to-all") happens on the host while sharding:
    core e receives the tokens routed to expert e (gathered, padded to a
    uniform capacity) plus expert e's weights, and a 1/8 slice of tokens
    for the (replicated) shared expert.
  - Each core runs two SwiGLU-style FFNs in bf16 on the tensor engine:
    its expert over `cap` gathered tokens and the shared expert over
    T/8 tokens.  Activations are kept in "transposed" [D, tokens]
    layout so no on-device transposes are needed.
  - Host un-shards: scatter-adds the scaled expert outputs (each token
    appears in exactly two cores' outputs) and writes shared outputs.

All matmuls contract over the partition dim in 128-chunks; weights are
pre-transposed and pre-tiled on the host so every DMA is per-partition
contiguous.
"""

import math
import numpy as np
import ml_dtypes

BF16 = ml_dtypes.bfloat16

# Problem dims (nn_FFNwMoE: T tokens, D model, H hidden, E experts, top-2)
T, D, H, E, TOPK = 4096, 1024, 2048, 8, 2
NCORES = 8
P = 128
KD = D // P   # 8  k-tiles over D
KH = H // P   # 16 k-tiles over H
NS = T // NCORES  # shared-expert tokens per core (512)
AUX_COEF = 0.01

# module-level stash so test.py can read profiling info
LAST_RESULT = None
_NC_CACHE = {}


def _ensure_axon_ntff_hook():
    """concourse.bass_utils imports antenv.axon_hooks when tracing under
    axon; some images lack that module.  Provide a minimal equivalent
    backed by the injected libaxon_pjrt.so so profiling degrades
    gracefully instead of raising ImportError."""
    try:
        import antenv.axon_hooks  # noqa: F401
        return
    except ImportError:
        pass
    import sys
    import types
    import contextlib
    import ctypes
    import os

    mod = types.ModuleType("antenv.axon_hooks")
    state = {"hook": None}

    def set_axon_ntff_profile_hook(h):
        state["hook"] = h

    def get_axon_ntff_profile_hook():
        return state["hook"]

    mod.set_axon_ntff_profile_hook = set_axon_ntff_profile_hook
    mod.get_axon_ntff_profile_hook = get_axon_ntff_profile_hook

    so_path = "/opt/axon/libaxon_pjrt.so"
    if os.path.exists(so_path):
        try:
            lib = ctypes.CDLL(so_path)
            if hasattr(lib, "axon_start_nrt_profile"):
                lib.axon_start_nrt_profile.argtypes = [
                    ctypes.POINTER(ctypes.c_int64), ctypes.c_size_t]
                lib.axon_start_nrt_profile.restype = ctypes.c_int64
                lib.axon_stop_nrt_profile.argtypes = [ctypes.c_char_p]
                lib.axon_stop_nrt_profile.restype = ctypes.c_int64

                @contextlib.contextmanager
                def _hook(output_dir, device_ids):
                    started = False
                    try:
                        import jax
                        jax.devices()
                        if device_ids:
                            ids = (ctypes.c_int64 * len(device_ids))(*device_ids)
                            rc = lib.axon_start_nrt_profile(ids, len(device_ids))
                        else:
                            rc = lib.axon_start_nrt_profile(None, 0)
                        started = rc == 0
                    except Exception:
                        pass
                    try:
                        yield
                    finally:
                        if started:
                            try:
                                n = lib.axon_stop_nrt_profile(
                                    str(output_dir).encode())
                                print(f"ntff profile: {n} file(s) -> {output_dir}")
                            except Exception:
                                pass

                state["hook"] = _hook
        except OSError:
            pass

    try:
        import antenv
        sys.modules["antenv.axon_hooks"] = mod
        antenv.axon_hooks = mod
    except ImportError:
        pass


def _n_slices(ncols, step=512):
    # remainder tile first: the kernel then *ends* on long 512-col PSUM
    # groups whose matmul runs hide the copy/DMA drain latency.
    out = []
    c = 0
    while c < ncols:
        out.append((c, min(step, ncols - c)))
        c += step
    if len(out) > 1 and out[-1][1] < step:
        out = [out[-1]] + out[:-1]
        # recompute offsets so the small tile is at columns [0, rem)
        sizes = [s for _, s in out]
        out = []
        c = 0
        for s in sizes:
            out.append((c, s))
            c += s
    return out


def _build_bass(cap, act="Gelu"):
    import concourse.bacc as bacc
    import concourse.tile as tile
    import concourse.mybir as mybir
    from contextlib import ExitStack

    dt = mybir.dt
    nc = bacc.Bacc("TRN2", target_bir_lowering=False)

    # Per-core DRAM I/O.  Layouts chosen so every DMA is per-partition
    # contiguous:
    #   activations: [P, KD, ncols]   (= xT[k*128+p, col])
    #   up weights:  [KH, P, KD, 128] (m-tile major)
    #   down weights:[KD, P, KH, 128] (d-tile major)
    xg = nc.dram_tensor("xg", [P, KD, cap], dt.bfloat16, kind="ExternalInput")
    xs = nc.dram_tensor("xs", [P, KD, NS], dt.bfloat16, kind="ExternalInput")
    w1e = nc.dram_tensor("w1e", [KH, P, KD, P], dt.bfloat16, kind="ExternalInput")
    w3e = nc.dram_tensor("w3e", [KH, P, KD, P], dt.bfloat16, kind="ExternalInput")
    w2e = nc.dram_tensor("w2e", [KD, P, KH, P], dt.bfloat16, kind="ExternalInput")
    w1s = nc.dram_tensor("w1s", [KH, P, KD, P], dt.bfloat16, kind="ExternalInput")
    w3s = nc.dram_tensor("w3s", [KH, P, KD, P], dt.bfloat16, kind="ExternalInput")
    w2s = nc.dram_tensor("w2s", [KD, P, KH, P], dt.bfloat16, kind="ExternalInput")
    yg = nc.dram_tensor("yg", [KD, P, cap], dt.float32, kind="ExternalOutput")
    ys = nc.dram_tensor("ys", [KD, P, NS], dt.float32, kind="ExternalOutput")

    GELU = getattr(mybir.ActivationFunctionType, act)
    MULT = mybir.AluOpType.mult

    with ExitStack() as ctx:
        tc = ctx.enter_context(tile.TileContext(nc))
        xpool = ctx.enter_context(tc.tile_pool(name="xp", bufs=1))
        hpool = ctx.enter_context(tc.tile_pool(name="hp", bufs=1))
        wpool = ctx.enter_context(tc.tile_pool(name="wp", bufs=10))
        gpool = ctx.enter_context(tc.tile_pool(name="gp", bufs=4))
        opool = ctx.enter_context(tc.tile_pool(name="op", bufs=3))
        ps_up = ctx.enter_context(tc.tile_pool(name="ps_up", bufs=4, space="PSUM"))
        ps_dn = ctx.enter_context(tc.tile_pool(name="ps_dn", bufs=4, space="PSUM"))

        # PE warmup: ~20 matmuls on zeroed scratch with no DMA deps, so the
        # HAM clock-gate reaches 8/8 during the preamble + first DMA wait
        # instead of half-clocking the first ~12us of real matmuls.
        w_warm = gpool.tile([P, P], dt.bfloat16, tag="wwarm", name="w_warm")
        x_warm = gpool.tile([P, 512], dt.bfloat16, tag="xwarm", name="x_warm")
        nc.vector.memset(w_warm[:], 0.0)
        nc.vector.memset(x_warm[:], 0.0)
        for _ in range(20):
            ps_w = ps_dn.tile([P, 512], dt.float32, tag="psd", name="ps_w")
            nc.tensor.matmul(ps_w, w_warm[:], x_warm[:], start=True, stop=True)

        def ffn(x_dram, w1_dram, w3_dram, w2_dram, y_dram, ncols, sfx):
            nsl_up = _n_slices(ncols)
            nsl = _n_slices(ncols)
            # First m-tile's weights BEFORE the activations: DMA issue is
            # serialized (~0.6us each), so order determines time-to-first-MM.
            w1_t0 = wpool.tile([P, KD, P], dt.bfloat16, tag="w13", name="w1_t0")
            nc.sync.dma_start(out=w1_t0[:], in_=w1_dram[0])
            x_sb = xpool.tile([P, KD, ncols], dt.bfloat16, tag=f"x{sfx}")
            nc.sync.dma_start(out=x_sb[:], in_=x_dram[:])
            h_sb = hpool.tile([P, KH, ncols], dt.bfloat16, tag=f"h{sfx}")
            # ---- up/gate proj + gelu + mul -> h_sb
            for m in range(KH):
                if m == 0:
                    w1_t = w1_t0
                else:
                    w1_t = wpool.tile([P, KD, P], dt.bfloat16, tag="w13")
                    nc.sync.dma_start(out=w1_t[:], in_=w1_dram[m])
                w3_t = wpool.tile([P, KD, P], dt.bfloat16, tag="w13")
                nc.sync.dma_start(out=w3_t[:], in_=w3_dram[m])
                for (n0, nsz) in nsl_up:
                    ps1 = ps_up.tile([P, 512], dt.float32, tag="psu", name="ps1")[:, :nsz]
                    ps3 = ps_up.tile([P, 512], dt.float32, tag="psu", name="ps3")[:, :nsz]
                    for k in range(KD):
                        nc.tensor.matmul(
                            ps1, w1_t[:, k], x_sb[:, k, n0:n0 + nsz],
                            start=(k == 0), stop=(k == KD - 1),
                        )
                    for k in range(KD):
                        nc.tensor.matmul(
                            ps3, w3_t[:, k], x_sb[:, k, n0:n0 + nsz],
                            start=(k == 0), stop=(k == KD - 1),
                        )
                    g = gpool.tile([P, 512], dt.float32, tag="g", name="g")[:, :nsz]
                    nc.scalar.activation(g, ps1, GELU)
                    nc.vector.tensor_tensor(h_sb[:, m, n0:n0 + nsz], g, ps3, MULT)
            # ---- down proj -> y
            for d in range(KD):
                w2_t = wpool.tile([P, KH, P], dt.bfloat16, tag="w2")
                nc.sync.dma_start(out=w2_t[:], in_=w2_dram[d])
                for (n0, nsz) in nsl:
                    ps = ps_dn.tile([P, 512], dt.float32, tag="psd", name="ps")[:, :nsz]
                    for k in range(KH):
                        nc.tensor.matmul(
                            ps, w2_t[:, k], h_sb[:, k, n0:n0 + nsz],
                            start=(k == 0), stop=(k == KH - 1),
                        )
                    o = opool.tile([P, 512], dt.float32, tag="o", name="o")[:, :nsz]
                    nc.scalar.copy(o, ps)
                    nc.sync.dma_start(out=y_dram[d][:, n0:n0 + nsz], in_=o)

        # Shared expert first: it needs only ~1.3 MB of DMA before its
        # matmuls start, hiding the larger routed-expert loads behind it.
        ffn(xs, w1s, w3s, w2s, ys, NS, "s")
        ffn(xg, w1e, w3e, w2e, yg, cap, "g")

    nc.compile()  # bacc passes: split multi-waits into event semaphores etc.
    return nc


def _tile_up_w(wT):
    # wT: [D, H] fp32 -> [KH, P, KD, P] bf16  (W[m,p,k,c] = wT[k*128+p, m*128+c])
    return np.ascontiguousarray(
        wT.reshape(KD, P, KH, P).transpose(2, 1, 0, 3)
    ).astype(BF16)


def _tile_dn_w(wT):
    # wT: [H, D] fp32 -> [KD, P, KH, P] bf16  (W[d,p,k,c] = wT[k*128+p, d*128+c])
    return np.ascontiguousarray(
        wT.reshape(KH, P, KD, P).transpose(2, 1, 0, 3)
    ).astype(BF16)


def _tile_act(xT, ncols):
    # xT: [D, ncols] fp32 -> [P, KD, ncols] bf16
    return np.ascontiguousarray(
        xT.reshape(KD, P, ncols).transpose(1, 0, 2)
    ).astype(BF16)


def kernel(x, Wr, w1, w2, w3, w1s, w2s, w3s):
    global LAST_RESULT
    _ensure_axon_ntff_hook()
    from concourse.bass_utils import run_bass_kernel_spmd

    x = np.asarray(x, dtype=np.float32)
    Wr = np.asarray(Wr, dtype=np.float32)
    w1 = np.asarray(w1, dtype=np.float32)
    w2 = np.asarray(w2, dtype=np.float32)
    w3 = np.asarray(w3, dtype=np.float32)
    w1s = np.asarray(w1s, dtype=np.float32)
    w2s = np.asarray(w2s, dtype=np.float32)
    w3s = np.asarray(w3s, dtype=np.float32)

    # ---------------- host router (fp32, matches reference math) ----------
    logits = x @ Wr.T                                    # [T, E]
    m1 = logits.max(axis=1, keepdims=True)
    ex = np.exp(logits - m1)
    probs = ex / ex.sum(axis=1, keepdims=True)
    order = np.argsort(-logits, axis=1, kind="stable")   # top-k order
    top1, top2 = order[:, 0], order[:, 1]
    density = np.bincount(top1, minlength=E).astype(np.float32) / np.float32(T)
    aux = np.float32(AUX_COEF * float((density * probs.mean(0)).sum()) * E)

    idx = [np.flatnonzero((top1 == e) | (top2 == e)) for e in range(E)]
    counts = [len(i) for i in idx]
    cap = max(256, int(math.ceil(max(counts) / 16.0)) * 16)

    # ---------------- shard inputs per core --------------------------------
    xT = x.T  # [D, T]
    w1s_t = _tile_up_w(w1s.T)
    w3s_t = _tile_up_w(w3s.T)
    w2s_t = _tile_dn_w(w2s.T)

    in_maps = []
    for e in range(E):
        xgT = np.zeros((D, cap), np.float32)
        xgT[:, :counts[e]] = xT[:, idx[e]]
        xsT = xT[:, e * NS:(e + 1) * NS]
        in_maps.append(dict(
            xg=_tile_act(xgT, cap),
            xs=_tile_act(np.ascontiguousarray(xsT), NS),
            w1e=_tile_up_w(w1[e].T),
            w3e=_tile_up_w(w3[e].T),
            w2e=_tile_dn_w(w2[e].T),
            w1s=w1s_t, w3s=w3s_t, w2s=w2s_t,
        ))

    # ---------------- compile + run on 8 cores -----------------------------
    if cap not in _NC_CACHE:
        _NC_CACHE[cap] = _build_bass(cap)
    nc = _NC_CACHE[cap]
    res = None
    last_exc = None
    for attempt in range(3):  # transient NRT exec-unit errors recover on retry
        try:
            res = run_bass_kernel_spmd(nc, in_maps, core_ids=list(range(NCORES)))
            break
        except Exception as e:  # noqa: BLE001
            last_exc = e
            import time
            time.sleep(2.0)
    if res is None:
        raise last_exc
    LAST_RESULT = res

    # ---------------- host un-shard / combine ------------------------------
    out = np.empty((T, D), np.float32)
    for e in range(E):
        ysT = res.results[e]["ys"].reshape(D, NS)        # [D, NS]
        out[e * NS:(e + 1) * NS] = ysT.T
    for e in range(E):
        ygT = res.results[e]["yg"].reshape(D, cap)       # [D, cap]
        ye = ygT.T[:counts[e]]                           # [cnt, D]
        out[idx[e]] += probs[idx[e], e][:, None] * ye
    return out, aux


# revision 42
# speedup vs baseline: 1.0301x; 1.0059x over previous
"""MoE FFN (8 experts, top-2, + shared expert) for 8 Trainium2 NeuronCores.

Strategy (expert-parallel, per sharding hint):
  - Host computes the (tiny) router: logits = x @ Wr.T, softmax, top-2,
    combine weights, aux loss.  This is 67 MFLOP of a ~155 GFLOP problem
    and is part of sharding/dispatch.
  - Token dispatch ("all-to-all") happens on the host while sharding:
    core e receives the tokens routed to expert e (gathered, padded to a
    uniform capacity) plus expert e's weights, and a 1/8 slice of tokens
    for the (replicated) shared expert.
  - Each core runs two SwiGLU-style FFNs in bf16 on the tensor engine:
    its expert over `cap` gathered tokens and the shared expert over
    T/8 tokens.  Activations are kept in "transposed" [D, tokens]
    layout so no on-device transposes are needed.
  - Host un-shards: scatter-adds the scaled expert outputs (each token
    appears in exactly two cores' outputs) and writes shared outputs.

All matmuls contract over the partition dim in 128-chunks; weights are
pre-transposed and pre-tiled on the host so every DMA is per-partition
contiguous.
"""

import math
import numpy as np
import ml_dtypes

BF16 = ml_dtypes.bfloat16

# Problem dims (nn_FFNwMoE: T tokens, D model, H hidden, E experts, top-2)
T, D, H, E, TOPK = 4096, 1024, 2048, 8, 2
NCORES = 8
P = 128
KD = D // P   # 8  k-tiles over D
KH = H // P   # 16 k-tiles over H
NS = T // NCORES  # shared-expert tokens per core (512)
AUX_COEF = 0.01

# module-level stash so test.py can read profiling info
LAST_RESULT = None
_NC_CACHE = {}


def _ensure_axon_ntff_hook():
    """concourse.bass_utils imports antenv.axon_hooks when tracing under
    axon; some images lack that module.  Provide a minimal equivalent
    backed by the injected libaxon_pjrt.so so profiling degrades
    gracefully instead of raising ImportError."""
    try:
        import antenv.axon_hooks  # noqa: F401
        return
    except ImportError:
        pass
    import sys
    import types
    import contextlib
    import ctypes
    import os

    mod = types.ModuleType("antenv.axon_hooks")
    state = {"hook": None}

    def set_axon_ntff_profile_hook(h):
        state["hook"] = h

    def get_axon_ntff_profile_hook():
        return state["hook"]

    mod.set_axon_ntff_profile_hook = set_axon_ntff_profile_hook
    mod.get_axon_ntff_profile_hook = get_axon_ntff_profile_hook

    so_path = "/opt/axon/libaxon_pjrt.so"
    if os.path.exists(so_path):
        try:
            lib = ctypes.CDLL(so_path)
            if hasattr(lib, "axon_start_nrt_profile"):
                lib.axon_start_nrt_profile.argtypes = [
                    ctypes.POINTER(ctypes.c_int64), ctypes.c_size_t]
                lib.axon_start_nrt_profile.restype = ctypes.c_int64
                lib.axon_stop_nrt_profile.argtypes = [ctypes.c_char_p]
                lib.axon_stop_nrt_profile.restype = ctypes.c_int64

                @contextlib.contextmanager
                def _hook(output_dir, device_ids):
                    started = False
                    try:
                        import jax
                        jax.devices()
                        if device_ids:
                            ids = (ctypes.c_int64 * len(device_ids))(*device_ids)
                            rc = lib.axon_start_nrt_profile(ids, len(device_ids))
                        else:
                            rc = lib.axon_start_nrt_profile(None, 0)
                        started = rc == 0
                    except Exception:
                        pass
                    try:
                        yield
                    finally:
                        if started:
                            try:
                                n = lib.axon_stop_nrt_profile(
                                    str(output_dir).encode())
                                print(f"ntff profile: {n} file(s) -> {output_dir}")
                            except Exception:
                                pass

                state["hook"] = _hook
        except OSError:
            pass

    try:
        import antenv
        sys.modules["antenv.axon_hooks"] = mod
        antenv.axon_hooks = mod
    except ImportError:
        pass


def _n_slices(ncols, step=512):
    # remainder tile first: the kernel then *ends* on long 512-col PSUM
    # groups whose matmul runs hide the copy/DMA drain latency.
    out = []
    c = 0
    while c < ncols:
        out.append((c, min(step, ncols - c)))
        c += step
    if len(out) > 1 and out[-1][1] < step:
        out = [out[-1]] + out[:-1]
        # recompute offsets so the small tile is at columns [0, rem)
        sizes = [s for _, s in out]
        out = []
        c = 0
        for s in sizes:
            out.append((c, s))
            c += s
    return out


def _build_bass(cap, act="Gelu"):
    import concourse.bacc as bacc
    import concourse.tile as tile
    import concourse.mybir as mybir
    from contextlib import ExitStack

    dt = mybir.dt
    nc = bacc.Bacc("TRN2", target_bir_lowering=False)

    # Per-core DRAM I/O.  Layouts chosen so every DMA is per-partition
    # contiguous:
    #   activations: [P, KD, ncols]   (= xT[k*128+p, col])
    #   up weights:  [KH, P, KD, 128] (m-tile major)
    #   down weights:[KD, P, KH, 128] (d-tile major)
    xg = nc.dram_tensor("xg", [P, KD, cap], dt.bfloat16, kind="ExternalInput")
    xs = nc.dram_tensor("xs", [P, KD, NS], dt.bfloat16, kind="ExternalInput")
    w1e = nc.dram_tensor("w1e", [KH, P, KD, P], dt.bfloat16, kind="ExternalInput")
    w3e = nc.dram_tensor("w3e", [KH, P, KD, P], dt.bfloat16, kind="ExternalInput")
    w2e = nc.dram_tensor("w2e", [KD, P, KH, P], dt.bfloat16, kind="ExternalInput")
    w1s = nc.dram_tensor("w1s", [KH, P, KD, P], dt.bfloat16, kind="ExternalInput")
    w3s = nc.dram_tensor("w3s", [KH, P, KD, P], dt.bfloat16, kind="ExternalInput")
    w2s = nc.dram_tensor("w2s", [KD, P, KH, P], dt.bfloat16, kind="ExternalInput")
    yg = nc.dram_tensor("yg", [KD, P, cap], dt.float32, kind="ExternalOutput")
    ys = nc.dram_tensor("ys", [KD, P, NS], dt.float32, kind="ExternalOutput")

    GELU = getattr(mybir.ActivationFunctionType, act)
    MULT = mybir.AluOpType.mult

    with ExitStack() as ctx:
        tc = ctx.enter_context(tile.TileContext(nc))
        xpool = ctx.enter_context(tc.tile_pool(name="xp", bufs=1))
        hpool = ctx.enter_context(tc.tile_pool(name="hp", bufs=1))
        wpool = ctx.enter_context(tc.tile_pool(name="wp", bufs=10))
        gpool = ctx.enter_context(tc.tile_pool(name="gp", bufs=6))
        opool = ctx.enter_context(tc.tile_pool(name="op", bufs=5))
        ps_up = ctx.enter_context(tc.tile_pool(name="ps_up", bufs=4, space="PSUM"))
        ps_dn = ctx.enter_context(tc.tile_pool(name="ps_dn", bufs=4, space="PSUM"))

        # PE warmup: ~20 matmuls on zeroed scratch with no DMA deps, so the
        # HAM clock-gate reaches 8/8 during the preamble + first DMA wait
        # instead of half-clocking the first ~12us of real matmuls.
        w_warm = gpool.tile([P, P], dt.bfloat16, tag="wwarm", name="w_warm")
        x_warm = gpool.tile([P, 512], dt.bfloat16, tag="xwarm", name="x_warm")
        nc.vector.memset(w_warm[:], 0.0)
        nc.vector.memset(x_warm[:], 0.0)
        for _ in range(20):
            ps_w = ps_dn.tile([P, 512], dt.float32, tag="psd", name="ps_w")
            nc.tensor.matmul(ps_w, w_warm[:], x_warm[:], start=True, stop=True)

        def ffn(x_dram, w1_dram, w3_dram, w2_dram, y_dram, ncols, sfx):
            nsl_up = _n_slices(ncols)
            nsl = _n_slices(ncols)
            # First m-tile's weights BEFORE the activations: DMA issue is
            # serialized (~0.6us each), so order determines time-to-first-MM.
            w1_t0 = wpool.tile([P, KD, P], dt.bfloat16, tag="w13", name="w1_t0")
            nc.sync.dma_start(out=w1_t0[:], in_=w1_dram[0])
            x_sb = xpool.tile([P, KD, ncols], dt.bfloat16, tag=f"x{sfx}")
            nc.sync.dma_start(out=x_sb[:], in_=x_dram[:])
            h_sb = hpool.tile([P, KH, ncols], dt.bfloat16, tag=f"h{sfx}")
            # ---- up/gate proj + gelu + mul -> h_sb
            for m in range(KH):
                if m == 0:
                    w1_t = w1_t0
                else:
                    w1_t = wpool.tile([P, KD, P], dt.bfloat16, tag="w13")
                    nc.sync.dma_start(out=w1_t[:], in_=w1_dram[m])
                w3_t = wpool.tile([P, KD, P], dt.bfloat16, tag="w13")
                nc.sync.dma_start(out=w3_t[:], in_=w3_dram[m])
                for (n0, nsz) in nsl_up:
                    ps1 = ps_up.tile([P, 512], dt.float32, tag="psu", name="ps1")[:, :nsz]
                    ps3 = ps_up.tile([P, 512], dt.float32, tag="psu", name="ps3")[:, :nsz]
                    for k in range(KD):
                        nc.tensor.matmul(
                            ps1, w1_t[:, k], x_sb[:, k, n0:n0 + nsz],
                            start=(k == 0), stop=(k == KD - 1),
                        )
                    for k in range(KD):
                        nc.tensor.matmul(
                            ps3, w3_t[:, k], x_sb[:, k, n0:n0 + nsz],
                            start=(k == 0), stop=(k == KD - 1),
                        )
                    g = gpool.tile([P, 512], dt.float32, tag="g", name="g")[:, :nsz]
                    nc.scalar.activation(g, ps1, GELU)
                    nc.vector.tensor_tensor(h_sb[:, m, n0:n0 + nsz], g, ps3, MULT)
            # ---- down proj -> y
            for d in range(KD):
                w2_t = wpool.tile([P, KH, P], dt.bfloat16, tag="w2")
                nc.sync.dma_start(out=w2_t[:], in_=w2_dram[d])
                for (n0, nsz) in nsl:
                    ps = ps_dn.tile([P, 512], dt.float32, tag="psd", name="ps")[:, :nsz]
                    for k in range(KH):
                        nc.tensor.matmul(
                            ps, w2_t[:, k], h_sb[:, k, n0:n0 + nsz],
                            start=(k == 0), stop=(k == KH - 1),
                        )
                    o = opool.tile([P, 512], dt.float32, tag="o", name="o")[:, :nsz]
                    nc.scalar.copy(o, ps)
                    nc.sync.dma_start(out=y_dram[d][:, n0:n0 + nsz], in_=o)

        # Shared expert first: it needs only ~1.3 MB of DMA before its
        # matmuls start, hiding the larger routed-expert loads behind it.
        ffn(xs, w1s, w3s, w2s, ys, NS, "s")
        ffn(xg, w1e, w3e, w2e, yg, cap, "g")

    nc.compile()  # bacc passes: split multi-waits into event semaphores etc.
    return nc


def _tile_up_w(wT):
    # wT: [D, H] fp32 -> [KH, P, KD, P] bf16  (W[m,p,k,c] = wT[k*128+p, m*128+c])
    return np.ascontiguousarray(
        wT.reshape(KD, P, KH, P).transpose(2, 1, 0, 3)
    ).astype(BF16)


def _tile_dn_w(wT):
    # wT: [H, D] fp32 -> [KD, P, KH, P] bf16  (W[d,p,k,c] = wT[k*128+p, d*128+c])
    return np.ascontiguousarray(
        wT.reshape(KH, P, KD, P).transpose(2, 1, 0, 3)
    ).astype(BF16)


def _tile_act(xT, ncols):
    # xT: [D, ncols] fp32 -> [P, KD, ncols] bf16
    return np.ascontiguousarray(
        xT.reshape(KD, P, ncols).transpose(1, 0, 2)
    ).astype(BF16)


def kernel(x, Wr, w1, w2, w3, w1s, w2s, w3s):
    global LAST_RESULT
    _ensure_axon_ntff_hook()
    from concourse.bass_utils import run_bass_kernel_spmd

    x = np.asarray(x, dtype=np.float32)
    Wr = np.asarray(Wr, dtype=np.float32)
    w1 = np.asarray(w1, dtype=np.float32)
    w2 = np.asarray(w2, dtype=np.float32)
    w3 = np.asarray(w3, dtype=np.float32)
    w1s = np.asarray(w1s, dtype=np.float32)
    w2s = np.asarray(w2s, dtype=np.float32)
    w3s = np.asarray(w3s, dtype=np.float32)

    # ---------------- host router (fp32, matches reference math) ----------
    logits = x @ Wr.T                                    # [T, E]
    m1 = logits.max(axis=1, keepdims=True)
    ex = np.exp(logits - m1)
    probs = ex / ex.sum(axis=1, keepdims=True)
    order = np.argsort(-logits, axis=1, kind="stable")   # top-k order
    top1, top2 = order[:, 0], order[:, 1]
    density = np.bincount(top1, minlength=E).astype(np.float32) / np.float32(T)
    aux = np.float32(AUX_COEF * float((density * probs.mean(0)).sum()) * E)

    idx = [np.flatnonzero((top1 == e) | (top2 == e)) for e in range(E)]
    counts = [len(i) for i in idx]
    cap = max(256, int(math.ceil(max(counts) / 16.0)) * 16)

    # ---------------- shard inputs per core --------------------------------
    xT = x.T  # [D, T]
    w1s_t = _tile_up_w(w1s.T)
    w3s_t = _tile_up_w(w3s.T)
    w2s_t = _tile_dn_w(w2s.T)

    in_maps = []
    for e in range(E):
        xgT = np.zeros((D, cap), np.float32)
        xgT[:, :counts[e]] = xT[:, idx[e]]
        xsT = xT[:, e * NS:(e + 1) * NS]
        in_maps.append(dict(
            xg=_tile_act(xgT, cap),
            xs=_tile_act(np.ascontiguousarray(xsT), NS),
            w1e=_tile_up_w(w1[e].T),
            w3e=_tile_up_w(w3[e].T),
            w2e=_tile_dn_w(w2[e].T),
            w1s=w1s_t, w3s=w3s_t, w2s=w2s_t,
        ))

    # ---------------- compile + run on 8 cores -----------------------------
    if cap not in _NC_CACHE:
        _NC_CACHE[cap] = _build_bass(cap)
    nc = _NC_CACHE[cap]
    res = None
    last_exc = None
    for attempt in range(3):  # transient NRT exec-unit errors recover on retry
        try:
            res = run_bass_kernel_spmd(nc, in_maps, core_ids=list(range(NCORES)))
            break
        except Exception as e:  # noqa: BLE001
            last_exc = e
            import time
            time.sleep(2.0)
    if res is None:
        raise last_exc
    LAST_RESULT = res

    # ---------------- host un-shard / combine ------------------------------
    out = np.empty((T, D), np.float32)
    for e in range(E):
        ysT = res.results[e]["ys"].reshape(D, NS)        # [D, NS]
        out[e * NS:(e + 1) * NS] = ysT.T
    for e in range(E):
        ygT = res.results[e]["yg"].reshape(D, cap)       # [D, cap]
        ye = ygT.T[:counts[e]]                           # [cnt, D]
        out[idx[e]] += probs[idx[e], e][:, None] * ye
    return out, aux
